# revision 31
# baseline (speedup 1.0000x reference)
"""Trainium2 Bass kernel for nn_DaleDendriticMLP (topk_masking).

Strategy: tensor-parallel over the 2048 hidden units across 8 NeuronCores
(256 units per core). Per layer, each core computes its shard's masked
feedforward + dendritic gating, extracts its local top-32 gated values per
sample, AllGathers the per-core sorted candidate lists fused with the
transposed activations, finds the exact per-row 102nd-largest value
(k-winners threshold), applies the mask, and feeds the next layer.

Precision: the k-winners ranking amplifies tiny numeric perturbations into
large output errors (a flipped winner routes through different output
weights), so matmuls must be f32-exact. The dendrite einsum (the PE
bottleneck) uses an exact fp16 hi/lo split: ctx = A + C, sw = H + L/2048
with A,B=fp16(ctx/2048),C and H,L all fp16, giving
d = A.H + B.L + C.H to ~1e-7 relative accuracy in 3 single-pass fp16
matmuls (vs 4 passes for native f32).

Scheduling: layer-2 dendrites are emitted between layer-1's AllGather and
threshold merge so the PE stays busy during the collective. Element-wise
work is split across Vector and GpSimd engines.

Host side does layout-only work: sharding, transposes, dtype casts and
exact hi/lo splits. All arithmetic (mask multiply, matmuls, gating, top-k,
Dale combine) runs on device.
"""

import os

os.environ.pop("JAX_PLATFORMS", None)
if os.environ.get("BASS_TRACE") != "1":
    os.environ["BASS_NEVER_TRACE"] = "1"

import numpy as np

import concourse.bacc as bacc
import concourse.tile as tile
import concourse.mybir as mybir
from concourse.bass_utils import run_bass_kernel_spmd

R = 8                    # cores
B = 256                  # batch
HID = 2048
U = HID // R             # 256 units per core
D_IN = 2048
D_CTX = 1024
KI = D_IN // 128         # 16 input K-chunks
KC = D_CTX // 128        # 8 context K-chunks
KH = HID // 128          # 16 hidden K-chunks
NSEG = 10
OUT = 100
KWIN = 102
LOC_ROUNDS = 4           # local top-32 per core
MERGE_ROUNDS = 13        # top-104 of merged 256
NEG = -1.0e30
SPLIT = 2048.0           # 2^11 hi/lo split scale

f32 = mybir.dt.float32
f16 = mybir.dt.float16
X = mybir.AxisListType.X
ALU = mybir.AluOpType
AF = mybir.ActivationFunctionType

_CACHE = {}
LAST_RESULT = None
ABL = set(x for x in os.environ.get("ABL", "").split(",") if x)


def _build(n_iters: int = 1):
    nc = bacc.Bacc(
        "TRN2",
        target_bir_lowering=False,
        debug=False,
        enable_asserts=True,
        num_devices=R,
    )

    dram = {}

    def din(name, shape, dt=f32):
        dram[name] = nc.dram_tensor(name, shape, dt, kind="ExternalInput")
        return dram[name]

    din("xT", [D_IN, B])
    for v in ("cA", "cB", "cC"):
        din(v, [D_CTX, B], f16)
    for L in (1, 2):
        din(f"wT{L}", [D_IN if L == 1 else HID, U])
        din(f"mwT{L}", [D_IN if L == 1 else HID, U], mybir.dt.bfloat16)
        din(f"sgH{L}", [D_CTX, 2, NSEG, 128], f16)
        din(f"sgL{L}", [D_CTX, 2, NSEG, 128], f16)
        din(f"msT{L}", [D_CTX, 2, NSEG, 128], f16)
        din(f"b{L}", [1, U])
    din("wexT", [HID, OUT + 1])   # col 100 = Wix
    din("weiT", [1, OUT])
    din("bout", [1, OUT])
    out_d = nc.dram_tensor("out", [B, OUT], f32, kind="ExternalOutput")

    ident_d = nc.inline_tensor(np.eye(128, dtype=np.float32), "ident")
    ones_d = nc.inline_tensor(np.ones((1, 128), np.float32), "ones_row")

    # One fused AllGather per layer: payload = yT shard (U*B) + top-32 lists
    PAY = U * B + B * 8 * LOC_ROUNDS          # 65536 + 8192 floats
    gath_g = {
        (L, i): nc.dram_tensor(f"gath_g{L}_{i}", [R * PAY], f32,
                               kind="Internal", addr_space="Shared")
        for L in (1, 2) for i in range(n_iters)
    }
    groups = [list(range(R))]

    with tile.TileContext(nc) as tc:
        with (
            tc.tile_pool(name="pa", bufs=1) as pa,          # persistent SBUF
            tc.tile_pool(name="pin", bufs=1) as pin,        # layer input (16KB)
            tc.tile_pool(name="pw", bufs=1) as pw,          # masked W (16KB)
            tc.tile_pool(name="pmw", bufs=2) as pmw,        # W-mask chunks
            tc.tile_pool(name="pseg", bufs=2) as pseg,      # masked seg fp16
            tc.tile_pool(name="pch", bufs=3) as pch,        # seg/mask raw chunks
            tc.tile_pool(name="pdram", bufs=1, space="DRAM") as pdram,
            tc.tile_pool(name="pp_y", bufs=1, space="PSUM") as pp_y,
            tc.tile_pool(name="pp_d", bufs=1, space="PSUM") as pp_d,
            tc.tile_pool(name="pp_m", bufs=1, space="PSUM") as pp_m,
        ):
            ident = pa.tile([128, 128], f32, tag="ident")
            nc.sync.dma_start(ident[:], ident_d[:])
            ones = pa.tile([1, 128], f32, tag="ones")
            nc.sync.dma_start(ones[:], ones_d[:])

            ctx3 = []
            for v in ("cA", "cB", "cC"):
                t = pa.tile([128, KC, B], f16, tag=v)
                nc.sync.dma_start(
                    t[:], dram[v][:].rearrange("(k p) b -> p k b", p=128))
                ctx3.append(t)

            # head weights, loaded once up front
            wex = pa.tile([128, KH, OUT + 1], f32, tag="wex")
            nc.sync.dma_start(
                wex[:], dram["wexT"][:].rearrange("(k p) o -> p k o", p=128))
            wei = pa.tile([1, OUT], f32, tag="wei")
            nc.sync.dma_start(wei[:], dram["weiT"][:])
            bo = pa.tile([1, OUT], f32, tag="bout")
            nc.sync.dma_start(bo[:], dram["bout"][:])

            def emit_prep(L):
                """Load + mask W; build masked seg (fp16 hi/lo); dendrite
                matmuls; segment max/min reduces."""
                nk = KI if L == 1 else KH
                wT_d, mwT_d = dram[f"wT{L}"], dram[f"mwT{L}"]

                wm = pw.tile([128, nk, U], f32, tag="wm")
                nc.sync.dma_start(wm[:], wT_d[:].rearrange("(k p) u -> p k u", p=128))
                for g4 in range(nk // 4):
                    mwc = pmw.tile([128, 4, U], mybir.dt.bfloat16, tag="mwc")
                    src = mwT_d[512 * g4:512 * (g4 + 1)]
                    nc.sync.dma_start(mwc[:], src.rearrange("(k p) u -> p k u", p=128))
                    nc.vector.tensor_tensor(
                        wm[:, 4 * g4:4 * (g4 + 1), :],
                        wm[:, 4 * g4:4 * (g4 + 1), :], mwc[:], op=ALU.mult)

                maxd = pa.tile([128, 2 * U], f32, tag="maxd")
                mind = pa.tile([128, 2 * U], f32, tag="mind")
                for uh in range(2):
                    smkH = pseg.tile([128, KC, NSEG * 128], f16, tag="smkH")
                    smkL = pseg.tile([128, KC, NSEG * 128], f16, tag="smkL")
                    for k in range(KC):
                        sgh = pch.tile([128, NSEG * 128], f16, tag="sgh")
                        nc.sync.dma_start(
                            sgh[:].rearrange("p (s u) -> p s u", s=NSEG),
                            dram[f"sgH{L}"][128 * k:128 * (k + 1), uh])
                        sgl = pch.tile([128, NSEG * 128], f16, tag="sgl")
                        nc.sync.dma_start(
                            sgl[:].rearrange("p (s u) -> p s u", s=NSEG),
                            dram[f"sgL{L}"][128 * k:128 * (k + 1), uh])
                        ms = pch.tile([128, NSEG * 128], f16, tag="ms")
                        nc.sync.dma_start(
                            ms[:].rearrange("p (s u) -> p s u", s=NSEG),
                            dram[f"msT{L}"][128 * k:128 * (k + 1), uh])
                        nc.vector.tensor_tensor(smkH[:, k, :], sgh[:], ms[:],
                                                op=ALU.mult)
                        nc.vector.tensor_tensor(smkL[:, k, :], sgl[:], ms[:],
                                                op=ALU.mult)
                    dps = [pp_d.tile([128, NSEG, 128], f32, tag=f"d{bt}",
                                     name=f"d{bt}") for bt in range(2)]
                    for bt in range(2):
                        dflat = dps[bt][:].rearrange("p s u -> p (s u)")
                        bs = slice(128 * bt, 128 * (bt + 1))
                        passes = [(ctx3[0], smkH), (ctx3[1], smkL),
                                  (ctx3[2], smkH)]
                        for k in range(KC):
                            for pi, (lhs, rhs) in enumerate(passes):
                                first = (k == 0 and pi == 0)
                                last = (k == KC - 1 and pi == 2)
                                for c0, ncols in ((0, 512), (512, 512),
                                                  (1024, 256)):
                                    nc.tensor.matmul(
                                        dflat[:, c0:c0 + ncols],
                                        lhsT=lhs[:, k, bs],
                                        rhs=rhs[:, k, c0:c0 + ncols],
                                        start=first, stop=last)
                    for bt in range(2):
                        v = dps[bt][:].rearrange("p s u -> p u s")
                        col = U * bt + 128 * uh
                        nc.vector.tensor_reduce(
                            maxd[:, col:col + 128], v, axis=X, op=ALU.max)
                        nc.vector.tensor_reduce(
                            mind[:, col:col + 128], v, axis=X, op=ALU.min)
                return wm, maxd, mind

            def emit_ff(L, it, in_sb, nk, wm, maxd, mind):
                """FF + gating + transpose + local top-k + payload write."""
                b_sb = pa.tile([1, U], f32, tag="bias")
                nc.sync.dma_start(b_sb[:], dram[f"b{L}"][:])

                y_all = pa.tile([128, 2 * U], f32, tag="y_all")
                for bt in range(2):
                    yp = pp_y.tile([128, U], f32, tag="yp")
                    for k in range(nk):
                        nc.tensor.matmul(
                            yp[:], lhsT=in_sb[:, k, 128 * bt:128 * (bt + 1)],
                            rhs=wm[:, k, :], start=(k == 0), stop=False)
                    nc.tensor.matmul(yp[:], lhsT=ones[:], rhs=b_sb[:],
                                     start=False, stop=True)
                    nc.scalar.copy(y_all[:, U * bt:U * (bt + 1)], yp[:])

                # abs-argmax gating: chosen = (maxd+mind>=0)?maxd:mind
                g = pa.tile([128, 2 * U], f32, tag="g")
                nc.vector.tensor_tensor(g[:], maxd[:], mind[:], op=ALU.add)
                gi = pa.tile([128, 2 * U], mybir.dt.uint8, tag="gi")
                nc.vector.tensor_scalar(gi[:], g[:], 0.0, None, op0=ALU.is_ge)
                chosen = pa.tile([128, 2 * U], f32, tag="chosen")
                nc.vector.tensor_copy(chosen[:], mind[:])
                nc.vector.copy_predicated(chosen[:], gi[:], maxd[:])
                sig = pa.tile([128, 2 * U], f32, tag="sig")
                nc.scalar.activation(sig[:], chosen[:], AF.Sigmoid)
                yg = pa.tile([128, 2 * U], f32, tag="yg")
                nc.vector.tensor_tensor(yg[:], y_all[:], sig[:], op=ALU.mult)

                # transpose yT shard (before top-k destroys yg)
                yT = pa.tile([128, 2, B], f32, tag="hT")
                for bt in range(2):
                    for j in range(2):
                        tp = pp_m.tile([128, 128], f32, tag="psm")
                        nc.tensor.transpose(
                            tp[:], yg[:, U * bt + 128 * j:U * bt + 128 * (j + 1)],
                            ident[:])
                        nc.scalar.copy(yT[:, j, 128 * bt:128 * (bt + 1)], tp[:])
                # local top-32 per row (destroys yg)
                vals = [pa.tile([128, 8 * LOC_ROUNDS], f32, tag=f"vals{bt}",
                                name=f"vals{bt}") for bt in range(2)]
                for bt in range(2):
                    sc = yg[:, U * bt:U * (bt + 1)]
                    for r in range(LOC_ROUNDS):
                        v8 = vals[bt][:, 8 * r:8 * (r + 1)]
                        nc.vector.max(v8, sc)
                        if r < LOC_ROUNDS - 1:
                            nc.vector.match_replace(sc, v8, sc, NEG)

                pay = pdram.tile([PAY], f32, tag="pay")
                nc.sync.dma_start(
                    pay[0:U * B].rearrange("(j p b) -> p j b", p=128, b=B), yT[:])
                for bt in range(2):
                    nc.sync.dma_start(
                        pay[U * B + 128 * 8 * LOC_ROUNDS * bt:
                            U * B + 128 * 8 * LOC_ROUNDS * (bt + 1)]
                        .rearrange("(p j) -> p j", p=128), vals[bt][:])
                return pay

            def emit_ag(L, it, pay):
                if "nocc" in ABL:
                    nc.sync.dma_start(gath_g[(L, it)][0:PAY], pay[:])
                else:
                    nc.gpsimd.collective_compute(
                        "AllGather", ALU.bypass, replica_groups=groups,
                        ins=[pay.opt()], outs=[gath_g[(L, it)][:]])

            def emit_back(L, it, out16=False):
                """Merge candidates -> exact per-row rank-102 threshold;
                load gathered yT; apply k-winners mask. out16: write the
                masked result to an fp16 tile (for the fp16 head)."""
                gath = gath_g[(L, it)]
                if "nomerge" in ABL:
                    thrc = pa.tile([128, 2], f32, tag="thrc")
                    nc.vector.memset(thrc[:], 0.5)
                    thr = [thrc[:, 0:1], thrc[:, 1:2]]
                else:
                    thr = []
                    for bt in range(2):
                        merged = pa.tile([128, R * 8 * LOC_ROUNDS], f32,
                                         tag="mrg", name=f"mrg{bt}")
                        src_ap = gath[:].rearrange(
                            "(r q) -> r q", q=PAY)[:, U * B:]
                        src_ap = src_ap.rearrange(
                            "r (p j) -> p r j", p=B)[128 * bt:128 * (bt + 1)]
                        nc.sync.dma_start(
                            merged[:].rearrange("p (r j) -> p r j", r=R), src_ap)
                        mv = pa.tile([128, 8 * MERGE_ROUNDS], f32,
                                     tag=f"mv{bt}", name=f"mv{bt}")
                        for r in range(MERGE_ROUNDS):
                            v8 = mv[:, 8 * r:8 * (r + 1)]
                            nc.vector.max(v8, merged[:])
                            if r < MERGE_ROUNDS - 1:
                                nc.vector.match_replace(merged[:], v8, merged[:], NEG)
                        thr.append(mv[:, KWIN - 1:KWIN])  # rank-102 value

                # broadcast thresholds across partitions: t_sb[p, b] = t[b]
                t_row = pa.tile([1, B], f32, tag="t_row")
                for bt in range(2):
                    tp = pp_m.tile([1, 128], f32, tag="psm")
                    nc.tensor.transpose(tp[:], thr[bt], ident[:])
                    nc.scalar.copy(t_row[:, 128 * bt:128 * (bt + 1)], tp[:])
                tbc = pp_m.tile([128, B], f32, tag="psm")
                nc.tensor.matmul(tbc[:], lhsT=ones[:], rhs=t_row[:],
                                 start=True, stop=True)
                t_sb = pa.tile([128, B], f32, tag="t_sb")
                nc.scalar.copy(t_sb[:], tbc[:])

                # load gathered yT; apply k-winners mask (split DVE/gpsimd)
                nxt = pin.tile([128, KH, B], f32, tag="xin")
                for r in range(R):
                    nc.sync.dma_start(
                        nxt[:, 2 * r:2 * (r + 1), :],
                        gath[r * PAY:r * PAY + U * B]
                        .rearrange("(j p b) -> p j b", p=128, b=B))
                h16 = None
                if out16:
                    h16 = pin.tile([128, KH, B], f16, tag="h16", name="h16")
                for k in range(KH):
                    # Pool writes fp16 incorrectly; keep fp16 outs on Vector
                    eng = nc.vector if (out16 or k % 2 == 0) else nc.gpsimd
                    msk = pa.tile([128, B], f32, tag=f"mskv{k % 2}",
                                  name="msk")
                    nc.vector.tensor_tensor(msk[:], nxt[:, k, :], t_sb[:],
                                            op=ALU.is_ge)
                    dst = h16[:, k, :] if out16 else nxt[:, k, :]
                    eng.tensor_tensor(dst, nxt[:, k, :], msk[:], op=ALU.mult)
                return h16 if out16 else nxt

            def emit_head(h2T):
                for bt in range(2):
                    zt = pp_y.tile([128, U], f32, tag="yp")
                    z = zt[:, 0:OUT + 1]
                    for k in range(KH):
                        nc.tensor.matmul(
                            z, lhsT=h2T[:, k, 128 * bt:128 * (bt + 1)],
                            rhs=wex[:, k, :], start=(k == 0), stop=(k == KH - 1))
                    zsb = pa.tile([128, OUT + 1], f32, tag="zsb")
                    nc.scalar.copy(zsb[:], z)
                    # Dale correction: out = z[:, :100] - z[:,100] * wei + bias
                    tp = pp_m.tile([1, 128], f32, tag="psm")
                    nc.tensor.transpose(tp[:], zsb[:, OUT:OUT + 1], ident[:])
                    nneg = pa.tile([1, 128], f32, tag="nneg")
                    nc.scalar.mul(nneg[:], tp[:], -1.0)
                    o2t = pp_y.tile([128, U], f32, tag="yp")
                    op2 = o2t[:, 0:OUT]
                    nc.tensor.matmul(op2, lhsT=nneg[:], rhs=wei[:],
                                     start=True, stop=False)
                    nc.tensor.matmul(op2, lhsT=ones[:], rhs=bo[:],
                                     start=False, stop=True)
                    ob = pa.tile([128, OUT], f32, tag="ob")
                    nc.vector.tensor_tensor(ob[:], zsb[:, 0:OUT], op2,
                                            op=ALU.add)
                    nc.sync.dma_start(out_d[128 * bt:128 * (bt + 1)], ob[:])

            for it in range(n_iters):
                xT = pin.tile([128, KI, B], f32, tag="xin0")
                nc.sync.dma_start(
                    xT[:], dram["xT"][:].rearrange("(k p) b -> p k b", p=128))
                wm1, maxd1, mind1 = emit_prep(1)
                pay1 = emit_ff(1, it, xT, KI, wm1, maxd1, mind1)
                wm2, maxd2, mind2 = emit_prep(2)
                emit_ag(1, it, pay1)
                h1 = emit_back(1, it)
                pay2 = emit_ff(2, it, h1, KH, wm2, maxd2, mind2)
                emit_ag(2, it, pay2)
                h2 = emit_back(2, it)
                emit_head(h2)

    nc.compile()
    return nc


def _prep_inputs(inputs):
    """Host-side layout-only prep: shard + transpose + exact fp16 hi/lo
    splits. Returns in_maps[8]."""
    import ml_dtypes  # noqa: F401  (bf16 dtype)
    np32 = lambda a: np.ascontiguousarray(np.asarray(a, dtype=np.float32))
    x = np32(inputs["x"]); ctx = np32(inputs["context"])
    ctxT = np.ascontiguousarray(ctx.T)
    wexT = np.concatenate(
        [np32(inputs["Wex_out"]).T, np32(inputs["Wix_out"]).T], axis=1)
    common = {
        "xT": np.ascontiguousarray(x.T),
        "cA": ctxT.astype(np.float16),
        "cB": (ctxT / SPLIT).astype(np.float16),
        "cC": (ctxT - ctxT.astype(np.float16).astype(np.float32)
               ).astype(np.float16),
        "wexT": np.ascontiguousarray(wexT),
        "weiT": np.ascontiguousarray(np32(inputs["Wei_out"]).T),
        "bout": np32(inputs["b_out"]).reshape(1, OUT),
    }
    in_maps = []
    for r in range(R):
        sh = slice(r * U, (r + 1) * U)
        m = dict(common)
        for L, (Wn, bn, sgn, mwn, msn) in {
            1: ("W1", "b1", "segW1", "maskW1", "maskS1"),
            2: ("W2", "b2", "segW2", "maskW2", "maskS2"),
        }.items():
            W = np32(inputs[Wn])[sh]          # [256, nin]
            mW = np32(inputs[mwn])[sh]
            sg = np32(inputs[sgn])[sh]        # [256, 10, 1024]
            msk = np32(inputs[msn])[sh]

            def seg_layout(a):
                # [u=256, s=10, c=1024] -> [c, uh=2, s, u128]
                t = a.transpose(2, 1, 0)                    # [c, s, u]
                t = t.reshape(D_CTX, NSEG, 2, 128)          # [c, s, uh, u]
                return np.ascontiguousarray(t.transpose(0, 2, 1, 3))

            sgT = seg_layout(sg)
            sgH = sgT.astype(np.float16)
            sgL = ((sgT - sgH.astype(np.float32)) * SPLIT).astype(np.float16)
            m[f"wT{L}"] = np.ascontiguousarray(W.T)
            m[f"mwT{L}"] = np.ascontiguousarray(mW.T).astype(ml_dtypes.bfloat16)
            m[f"sgH{L}"] = sgH
            m[f"sgL{L}"] = sgL
            m[f"msT{L}"] = seg_layout(msk).astype(np.float16)
            m[f"b{L}"] = np32(inputs[bn])[sh].reshape(1, U)
        in_maps.append(m)
    return in_maps


def kernel(**inputs) -> np.ndarray:
    global LAST_RESULT
    if "nc" not in _CACHE:
        _CACHE["nc"] = _build()
    in_maps = _prep_inputs(inputs)
    res = run_bass_kernel_spmd(_CACHE["nc"], in_maps, core_ids=list(range(R)))
    LAST_RESULT = res
    return np.asarray(res.results[0]["out"], dtype=np.float32)


# revision 33
# speedup vs baseline: 1.0377x; 1.0377x over previous
"""Trainium2 Bass kernel for nn_DaleDendriticMLP (topk_masking).

Strategy: tensor-parallel over the 2048 hidden units across 8 NeuronCores
(256 units per core).

Layer 1: each core computes its shard's masked feedforward + dendritic
gating, extracts its local top-32 gated values per sample, then issues TWO
AllGathers: a small one with the sorted candidate lists (32KB, so the
exact rank-102 threshold merge can start early) and a large one with the
transposed activations (256KB, which the merge overlaps). The k-winners
mask is applied to the gathered full h1.

Layer 2: the full h2 is never materialized. Only the candidate lists are
AllGathered (32KB); each core applies the threshold to its local shard,
computes its partial Dale head output (contraction over its 256 units),
and an AllReduce (100KB) sums the partials. This avoids the expensive
256KB-per-core activation gather for the last layer.

Precision: the k-winners ranking amplifies tiny numeric perturbations into
large output errors (a flipped winner routes through different output
weights), so matmuls must be f32-exact. The dendrite einsum (the PE
bottleneck) uses an exact fp16 hi/lo split: ctx ~ A + C and
sw ~ H + L/2048 (A=fp16(ctx), B=fp16(ctx/2048), C=fp16(ctx-A),
H=fp16(sw), L=fp16((sw-H)*2048)), giving d = A.H + B.L + C.H to ~1e-7
relative accuracy in 3 single-pass fp16 matmuls (vs 4 passes for native
f32 matmul).

Scheduling: layer-2 dendrites are emitted between layer-1's AllGathers and
threshold merge so the PE stays busy during the collectives.

Host side does layout-only work: sharding, transposes, dtype casts and
exact fp16 hi/lo splits. All arithmetic (mask multiply, matmuls, gating,
top-k, Dale combine) runs on device.
"""

import os

os.environ.pop("JAX_PLATFORMS", None)
if os.environ.get("BASS_TRACE") != "1":
    os.environ["BASS_NEVER_TRACE"] = "1"

import numpy as np

import concourse.bacc as bacc
import concourse.tile as tile
import concourse.mybir as mybir
from concourse.bass_utils import run_bass_kernel_spmd

R = 8                    # cores
B = 256                  # batch
HID = 2048
U = HID // R             # 256 units per core
D_IN = 2048
D_CTX = 1024
KI = D_IN // 128         # 16 input K-chunks
KC = D_CTX // 128        # 8 context K-chunks
KH = HID // 128          # 16 hidden K-chunks
NSEG = 10
OUT = 100
KWIN = 102
LOC_ROUNDS = 4           # local top-32 per core
MERGE_ROUNDS = 13        # top-104 of merged 256
NEG = -1.0e30
SPLIT = 2048.0           # 2^11 hi/lo split scale
NV = B * 8 * LOC_ROUNDS  # vals payload floats (8192)
NY = U * B               # yT payload floats (65536)

f32 = mybir.dt.float32
f16 = mybir.dt.float16
X = mybir.AxisListType.X
ALU = mybir.AluOpType
AF = mybir.ActivationFunctionType

_CACHE = {}
LAST_RESULT = None
ABL = set(x for x in os.environ.get("ABL", "").split(",") if x)


def _build(n_iters: int = 1):
    nc = bacc.Bacc(
        "TRN2",
        target_bir_lowering=False,
        debug=False,
        enable_asserts=True,
        num_devices=R,
    )

    dram = {}

    def din(name, shape, dt=f32):
        dram[name] = nc.dram_tensor(name, shape, dt, kind="ExternalInput")
        return dram[name]

    din("xT", [D_IN, B])
    for v in ("cA", "cB", "cC"):
        din(v, [D_CTX, B], f16)
    for L in (1, 2):
        din(f"wT{L}", [D_IN if L == 1 else HID, U])
        din(f"mwT{L}", [D_IN if L == 1 else HID, U], mybir.dt.bfloat16)
        din(f"sgH{L}", [D_CTX, 2, NSEG, 128], f16)
        din(f"sgL{L}", [D_CTX, 2, NSEG, 128], f16)
        din(f"msT{L}", [D_CTX, 2, NSEG, 128], f16)
        din(f"b{L}", [1, U])
    din("wexS", [U, OUT + 1])     # per-core W_out shard; col 100 = Wix
    din("weiT", [1, OUT])
    din("bout", [1, OUT])
    out_d = nc.dram_tensor("out", [B, OUT], f32, kind="ExternalOutput")

    ident_d = nc.inline_tensor(np.eye(128, dtype=np.float32), "ident")
    ones_d = nc.inline_tensor(np.ones((1, 128), np.float32), "ones_row")

    gath_v = {
        (L, i): nc.dram_tensor(f"gath_v{L}_{i}", [R * NV], f32,
                               kind="Internal", addr_space="Shared")
        for L in (1, 2) for i in range(n_iters)
    }
    gath_y = {
        i: nc.dram_tensor(f"gath_y1_{i}", [R * NY], f32,
                          kind="Internal", addr_space="Shared")
        for i in range(n_iters)
    }
    ar_out = {
        i: nc.dram_tensor(f"ar_out_{i}", [B * OUT], f32,
                          kind="Internal", addr_space="Shared")
        for i in range(n_iters)
    }
    groups = [list(range(R))]

    with tile.TileContext(nc) as tc:
        with (
            tc.tile_pool(name="pa", bufs=1) as pa,          # persistent SBUF
            tc.tile_pool(name="pin", bufs=1) as pin,        # layer input (16KB)
            tc.tile_pool(name="pw", bufs=1) as pw,          # masked W (16KB)
            tc.tile_pool(name="pmw", bufs=2) as pmw,        # W-mask chunks
            tc.tile_pool(name="pseg", bufs=2) as pseg,      # masked seg fp16
            tc.tile_pool(name="pch", bufs=3) as pch,        # seg/mask raw chunks
            tc.tile_pool(name="pdram", bufs=1, space="DRAM") as pdram,
            tc.tile_pool(name="pp_y", bufs=1, space="PSUM") as pp_y,
            tc.tile_pool(name="pp_d", bufs=1, space="PSUM") as pp_d,
            tc.tile_pool(name="pp_m", bufs=1, space="PSUM") as pp_m,
        ):
            ident = pa.tile([128, 128], f32, tag="ident")
            nc.sync.dma_start(ident[:], ident_d[:])
            ones = pa.tile([1, 128], f32, tag="ones")
            nc.sync.dma_start(ones[:], ones_d[:])

            ctx3 = []
            for v in ("cA", "cB", "cC"):
                t = pa.tile([128, KC, B], f16, tag=v)
                nc.sync.dma_start(
                    t[:], dram[v][:].rearrange("(k p) b -> p k b", p=128))
                ctx3.append(t)

            # head weights (local shard), loaded once up front
            wexs = pa.tile([128, 2, OUT + 1], f32, tag="wexs")
            nc.sync.dma_start(
                wexs[:], dram["wexS"][:].rearrange("(j p) o -> p j o", p=128))
            wei = pa.tile([1, OUT], f32, tag="wei")
            nc.sync.dma_start(wei[:], dram["weiT"][:])
            bo = pa.tile([1, OUT], f32, tag="bout")
            nc.sync.dma_start(bo[:], dram["bout"][:])
            # bias broadcast across partitions (for the post-AllReduce add)
            bbc_ps = pp_m.tile([128, OUT], f32, tag="psm", name="bbc_ps")
            nc.tensor.matmul(bbc_ps[:], lhsT=ones[:], rhs=bo[:],
                             start=True, stop=True)
            bias_bc = pa.tile([128, OUT], f32, tag="bias_bc")
            nc.scalar.copy(bias_bc[:], bbc_ps[:])

            def emit_prep(L):
                """Load + mask W; build masked seg (fp16 hi/lo); dendrite
                matmuls (fp16 3-pass split); segment max/min reduces."""
                nk = KI if L == 1 else KH
                wT_d, mwT_d = dram[f"wT{L}"], dram[f"mwT{L}"]

                wm = pw.tile([128, nk, U], f32, tag="wm")
                nc.sync.dma_start(wm[:], wT_d[:].rearrange("(k p) u -> p k u", p=128))
                for g4 in range(nk // 4):
                    mwc = pmw.tile([128, 4, U], mybir.dt.bfloat16, tag="mwc")
                    src = mwT_d[512 * g4:512 * (g4 + 1)]
                    nc.sync.dma_start(mwc[:], src.rearrange("(k p) u -> p k u", p=128))
                    nc.vector.tensor_tensor(
                        wm[:, 4 * g4:4 * (g4 + 1), :],
                        wm[:, 4 * g4:4 * (g4 + 1), :], mwc[:], op=ALU.mult)

                maxd = pa.tile([128, 2 * U], f32, tag="maxd")
                mind = pa.tile([128, 2 * U], f32, tag="mind")
                for uh in range(2):
                    smkH = pseg.tile([128, KC, NSEG * 128], f16, tag="smkH")
                    smkL = pseg.tile([128, KC, NSEG * 128], f16, tag="smkL")
                    for k in range(KC):
                        sgh = pch.tile([128, NSEG * 128], f16, tag="sgh")
                        nc.sync.dma_start(
                            sgh[:].rearrange("p (s u) -> p s u", s=NSEG),
                            dram[f"sgH{L}"][128 * k:128 * (k + 1), uh])
                        sgl = pch.tile([128, NSEG * 128], f16, tag="sgl")
                        nc.sync.dma_start(
                            sgl[:].rearrange("p (s u) -> p s u", s=NSEG),
                            dram[f"sgL{L}"][128 * k:128 * (k + 1), uh])
                        ms = pch.tile([128, NSEG * 128], f16, tag="ms")
                        nc.sync.dma_start(
                            ms[:].rearrange("p (s u) -> p s u", s=NSEG),
                            dram[f"msT{L}"][128 * k:128 * (k + 1), uh])
                        nc.vector.tensor_tensor(smkH[:, k, :], sgh[:], ms[:],
                                                op=ALU.mult)
                        nc.vector.tensor_tensor(smkL[:, k, :], sgl[:], ms[:],
                                                op=ALU.mult)
                    dps = [pp_d.tile([128, NSEG, 128], f32, tag=f"d{bt}",
                                     name=f"d{bt}") for bt in range(2)]
                    for bt in range(2):
                        dflat = dps[bt][:].rearrange("p s u -> p (s u)")
                        bs = slice(128 * bt, 128 * (bt + 1))
                        passes = [(ctx3[0], smkH), (ctx3[1], smkL),
                                  (ctx3[2], smkH)]
                        for k in range(KC):
                            for pi, (lhs, rhs) in enumerate(passes):
                                first = (k == 0 and pi == 0)
                                last = (k == KC - 1 and pi == 2)
                                for c0, ncols in ((0, 512), (512, 512),
                                                  (1024, 256)):
                                    nc.tensor.matmul(
                                        dflat[:, c0:c0 + ncols],
                                        lhsT=lhs[:, k, bs],
                                        rhs=rhs[:, k, c0:c0 + ncols],
                                        start=first, stop=last)
                    for bt in range(2):
                        v = dps[bt][:].rearrange("p s u -> p u s")
                        col = U * bt + 128 * uh
                        nc.vector.tensor_reduce(
                            maxd[:, col:col + 128], v, axis=X, op=ALU.max)
                        nc.vector.tensor_reduce(
                            mind[:, col:col + 128], v, axis=X, op=ALU.min)
                return wm, maxd, mind

            def emit_ff(L, it, in_sb, nk, wm, maxd, mind):
                """FF + gating + (L1: transpose) + local top-k + payload
                writes. Returns (pay_v, y_all, sig)."""
                b_sb = pa.tile([1, U], f32, tag="bias")
                nc.sync.dma_start(b_sb[:], dram[f"b{L}"][:])

                y_all = pa.tile([128, 2 * U], f32, tag="y_all")
                for bt in range(2):
                    yp = pp_y.tile([128, U], f32, tag="yp")
                    for k in range(nk):
                        nc.tensor.matmul(
                            yp[:], lhsT=in_sb[:, k, 128 * bt:128 * (bt + 1)],
                            rhs=wm[:, k, :], start=(k == 0), stop=False)
                    nc.tensor.matmul(yp[:], lhsT=ones[:], rhs=b_sb[:],
                                     start=False, stop=True)
                    nc.scalar.copy(y_all[:, U * bt:U * (bt + 1)], yp[:])

                # abs-argmax gating: chosen = (maxd+mind>=0)?maxd:mind
                g = pa.tile([128, 2 * U], f32, tag="g")
                nc.vector.tensor_tensor(g[:], maxd[:], mind[:], op=ALU.add)
                gi = pa.tile([128, 2 * U], mybir.dt.uint8, tag="gi")
                nc.vector.tensor_scalar(gi[:], g[:], 0.0, None, op0=ALU.is_ge)
                chosen = pa.tile([128, 2 * U], f32, tag="chosen")
                nc.vector.tensor_copy(chosen[:], mind[:])
                nc.vector.copy_predicated(chosen[:], gi[:], maxd[:])
                sig = pa.tile([128, 2 * U], f32, tag="sig")
                nc.scalar.activation(sig[:], chosen[:], AF.Sigmoid)
                yg = pa.tile([128, 2 * U], f32, tag="yg")
                nc.vector.tensor_tensor(yg[:], y_all[:], sig[:], op=ALU.mult)

                if L == 1:
                    # transpose yT shard (before top-k destroys yg)
                    yT = pa.tile([128, 2, B], f32, tag="hT")
                    for bt in range(2):
                        for j in range(2):
                            tp = pp_m.tile([128, 128], f32, tag="psm")
                            nc.tensor.transpose(
                                tp[:],
                                yg[:, U * bt + 128 * j:U * bt + 128 * (j + 1)],
                                ident[:])
                            nc.scalar.copy(yT[:, j, 128 * bt:128 * (bt + 1)],
                                           tp[:])

                # local top-32 per row (destroys yg)
                vals = [pa.tile([128, 8 * LOC_ROUNDS], f32, tag=f"vals{bt}",
                                name=f"vals{bt}") for bt in range(2)]
                for bt in range(2):
                    sc = yg[:, U * bt:U * (bt + 1)]
                    for r in range(LOC_ROUNDS):
                        v8 = vals[bt][:, 8 * r:8 * (r + 1)]
                        nc.vector.max(v8, sc)
                        if r < LOC_ROUNDS - 1:
                            nc.vector.match_replace(sc, v8, sc, NEG)

                pay_v = pdram.tile([NV], f32, tag="pay_v")
                for bt in range(2):
                    nc.sync.dma_start(
                        pay_v[128 * 8 * LOC_ROUNDS * bt:
                              128 * 8 * LOC_ROUNDS * (bt + 1)]
                        .rearrange("(p j) -> p j", p=128), vals[bt][:])
                if L == 1:
                    pay_y = pdram.tile([NY], f32, tag="pay_y")
                    nc.sync.dma_start(
                        pay_y[:].rearrange("(j p b) -> p j b", p=128, b=B),
                        yT[:])
                else:
                    pay_y = None
                return pay_v, pay_y, y_all, sig

            def emit_ag(dst, pay):
                if "nocc" in ABL:
                    nc.sync.dma_start(dst[0:pay[:].size()], pay[:])
                else:
                    nc.gpsimd.collective_compute(
                        "AllGather", ALU.bypass, replica_groups=groups,
                        ins=[pay.opt()], outs=[dst[:]])

            def emit_merge(gv):
                """Exact per-row rank-102 threshold from gathered top-32
                lists. Returns thr[bt] as per-partition [128,1] APs."""
                thr = []
                for bt in range(2):
                    merged = pa.tile([128, R * 8 * LOC_ROUNDS], f32,
                                     tag="mrg", name=f"mrg{bt}")
                    src_ap = gv[:].rearrange("(r q) -> r q", q=NV)
                    src_ap = src_ap.rearrange(
                        "r (p j) -> p r j", p=B)[128 * bt:128 * (bt + 1)]
                    nc.sync.dma_start(
                        merged[:].rearrange("p (r j) -> p r j", r=R), src_ap)
                    mv = pa.tile([128, 8 * MERGE_ROUNDS], f32,
                                 tag=f"mv{bt}", name=f"mv{bt}")
                    for r in range(MERGE_ROUNDS):
                        v8 = mv[:, 8 * r:8 * (r + 1)]
                        nc.vector.max(v8, merged[:])
                        if r < MERGE_ROUNDS - 1:
                            nc.vector.match_replace(merged[:], v8, merged[:],
                                                    NEG)
                    thr.append(mv[:, KWIN - 1:KWIN])  # rank-102 value
                return thr

            def emit_back1(it, thr):
                """Broadcast thresholds; load gathered yT; apply k-winners
                mask -> full h1 [128, KH, B]."""
                gy = gath_y[it]
                t_row = pa.tile([1, B], f32, tag="t_row")
                for bt in range(2):
                    tp = pp_m.tile([1, 128], f32, tag="psm")
                    nc.tensor.transpose(tp[:], thr[bt], ident[:])
                    nc.scalar.copy(t_row[:, 128 * bt:128 * (bt + 1)], tp[:])
                tbc = pp_m.tile([128, B], f32, tag="psm")
                nc.tensor.matmul(tbc[:], lhsT=ones[:], rhs=t_row[:],
                                 start=True, stop=True)
                t_sb = pa.tile([128, B], f32, tag="t_sb")
                nc.scalar.copy(t_sb[:], tbc[:])

                nxt = pin.tile([128, KH, B], f32, tag="xin")
                for r in range(R):
                    nc.sync.dma_start(
                        nxt[:, 2 * r:2 * (r + 1), :],
                        gy[r * NY:(r + 1) * NY]
                        .rearrange("(j p b) -> p j b", p=128, b=B))
                for k in range(KH):
                    eng = nc.vector if k % 2 == 0 else nc.gpsimd
                    msk = pa.tile([128, B], f32, tag=f"mskv{k % 2}",
                                  name="msk")
                    nc.vector.tensor_tensor(msk[:], nxt[:, k, :], t_sb[:],
                                            op=ALU.is_ge)
                    eng.tensor_tensor(nxt[:, k, :], nxt[:, k, :], msk[:],
                                      op=ALU.mult)
                return nxt

            def emit_tail(it, thr, y_all, sig):
                """Local k-winners mask on own shard, partial Dale head,
                AllReduce, bias, output."""
                h2T = pa.tile([128, 2, B], f32, tag="hT")
                for bt in range(2):
                    cs = slice(U * bt, U * (bt + 1))
                    ygm = pa.tile([128, U], f32, tag="ygm", name="ygm")
                    nc.vector.tensor_tensor(ygm[:], y_all[:, cs], sig[:, cs],
                                            op=ALU.mult)
                    mskl = pa.tile([128, U], f32, tag="mskl", name="mskl")
                    nc.vector.tensor_scalar(mskl[:], ygm[:], thr[bt], None,
                                            op0=ALU.is_ge)
                    nc.vector.tensor_tensor(ygm[:], ygm[:], mskl[:],
                                            op=ALU.mult)
                    for j in range(2):
                        tp = pp_m.tile([128, 128], f32, tag="psm")
                        nc.tensor.transpose(
                            tp[:], ygm[:, 128 * j:128 * (j + 1)], ident[:])
                        nc.scalar.copy(h2T[:, j, 128 * bt:128 * (bt + 1)],
                                       tp[:])

                ar_in = pdram.tile([B * OUT], f32, tag="ar_in")
                for bt in range(2):
                    zt = pp_y.tile([128, U], f32, tag="yp")
                    z = zt[:, 0:OUT + 1]
                    for j in range(2):
                        nc.tensor.matmul(
                            z, lhsT=h2T[:, j, 128 * bt:128 * (bt + 1)],
                            rhs=wexs[:, j, :], start=(j == 0), stop=(j == 1))
                    zsb = pa.tile([128, OUT + 1], f32, tag="zsb")
                    nc.scalar.copy(zsb[:], z)
                    # Dale correction: oloc = z[:, :100] - z[:,100] * wei
                    tp = pp_m.tile([1, 128], f32, tag="psm")
                    nc.tensor.transpose(tp[:], zsb[:, OUT:OUT + 1], ident[:])
                    nneg = pa.tile([1, 128], f32, tag="nneg")
                    nc.scalar.mul(nneg[:], tp[:], -1.0)
                    o2t = pp_y.tile([128, U], f32, tag="yp")
                    op2 = o2t[:, 0:OUT]
                    nc.tensor.matmul(op2, lhsT=nneg[:], rhs=wei[:],
                                     start=True, stop=True)
                    obl = pa.tile([128, OUT], f32, tag="obl")
                    nc.vector.tensor_tensor(obl[:], zsb[:, 0:OUT], op2,
                                            op=ALU.add)
                    nc.sync.dma_start(
                        ar_in[:].rearrange("(b o) -> b o", o=OUT)
                        [128 * bt:128 * (bt + 1)], obl[:])

                aro = ar_out[it]
                if "nocc" in ABL:
                    nc.sync.dma_start(aro[:], ar_in[:])
                else:
                    nc.gpsimd.collective_compute(
                        "AllReduce", ALU.add, replica_groups=groups,
                        ins=[ar_in.opt()], outs=[aro[:]])
                for bt in range(2):
                    osb = pa.tile([128, OUT], f32, tag="osb")
                    nc.sync.dma_start(
                        osb[:], aro[:].rearrange("(b o) -> b o", o=OUT)
                        [128 * bt:128 * (bt + 1)])
                    nc.vector.tensor_tensor(osb[:], osb[:], bias_bc[:],
                                            op=ALU.add)
                    nc.sync.dma_start(out_d[128 * bt:128 * (bt + 1)], osb[:])

            for it in range(n_iters):
                xT = pin.tile([128, KI, B], f32, tag="xin0")
                nc.sync.dma_start(
                    xT[:], dram["xT"][:].rearrange("(k p) b -> p k b", p=128))
                wm1, maxd1, mind1 = emit_prep(1)
                pv1, py1, _, _ = emit_ff(1, it, xT, KI, wm1, maxd1, mind1)
                wm2, maxd2, mind2 = emit_prep(2)
                emit_ag(gath_v[(1, it)], pv1)
                emit_ag(gath_y[it], py1)
                thr1 = emit_merge(gath_v[(1, it)])
                h1 = emit_back1(it, thr1)
                pv2, _, y_all2, sig2 = emit_ff(2, it, h1, KH, wm2, maxd2,
                                               mind2)
                emit_ag(gath_v[(2, it)], pv2)
                thr2 = emit_merge(gath_v[(2, it)])
                emit_tail(it, thr2, y_all2, sig2)

    nc.compile()
    return nc


def _prep_inputs(inputs):
    """Host-side layout-only prep: shard + transpose + exact fp16 hi/lo
    splits. Returns in_maps[8]."""
    import ml_dtypes
    np32 = lambda a: np.ascontiguousarray(np.asarray(a, dtype=np.float32))
    x = np32(inputs["x"]); ctx = np32(inputs["context"])
    ctxT = np.ascontiguousarray(ctx.T)
    wexT = np.concatenate(
        [np32(inputs["Wex_out"]).T, np32(inputs["Wix_out"]).T], axis=1)
    common = {
        "xT": np.ascontiguousarray(x.T),
        "cA": ctxT.astype(np.float16),
        "cB": (ctxT / SPLIT).astype(np.float16),
        "cC": (ctxT - ctxT.astype(np.float16).astype(np.float32)
               ).astype(np.float16),
        "weiT": np.ascontiguousarray(np32(inputs["Wei_out"]).T),
        "bout": np32(inputs["b_out"]).reshape(1, OUT),
    }
    in_maps = []
    for r in range(R):
        sh = slice(r * U, (r + 1) * U)
        m = dict(common)
        m["wexS"] = np.ascontiguousarray(wexT[sh])      # [256, 101]
        for L, (Wn, bn, sgn, mwn, msn) in {
            1: ("W1", "b1", "segW1", "maskW1", "maskS1"),
            2: ("W2", "b2", "segW2", "maskW2", "maskS2"),
        }.items():
            W = np32(inputs[Wn])[sh]          # [256, nin]
            mW = np32(inputs[mwn])[sh]
            sg = np32(inputs[sgn])[sh]        # [256, 10, 1024]
            msk = np32(inputs[msn])[sh]

            def seg_layout(a):
                # [u=256, s=10, c=1024] -> [c, uh=2, s, u128]
                t = a.transpose(2, 1, 0)                    # [c, s, u]
                t = t.reshape(D_CTX, NSEG, 2, 128)          # [c, s, uh, u]
                return np.ascontiguousarray(t.transpose(0, 2, 1, 3))

            sgT = seg_layout(sg)
            sgH = sgT.astype(np.float16)
            sgL = ((sgT - sgH.astype(np.float32)) * SPLIT).astype(np.float16)
            m[f"wT{L}"] = np.ascontiguousarray(W.T)
            m[f"mwT{L}"] = np.ascontiguousarray(mW.T).astype(ml_dtypes.bfloat16)
            m[f"sgH{L}"] = sgH
            m[f"sgL{L}"] = sgL
            m[f"msT{L}"] = seg_layout(msk).astype(np.float16)
            m[f"b{L}"] = np32(inputs[bn])[sh].reshape(1, U)
        in_maps.append(m)
    return in_maps


def kernel(**inputs) -> np.ndarray:
    global LAST_RESULT
    if "nc" not in _CACHE:
        _CACHE["nc"] = _build()
    in_maps = _prep_inputs(inputs)
    res = run_bass_kernel_spmd(_CACHE["nc"], in_maps, core_ids=list(range(R)))
    LAST_RESULT = res
    return np.asarray(res.results[0]["out"], dtype=np.float32)
